# revision 6
# baseline (speedup 1.0000x reference)
"""Trainium2 Bass kernel for gnn_message_passing (nn_Model_50225347559738).

Math: the reference computes, per (item n, neighbor slot k) with entity id
e = item_entities[n,k] and relation id r = item_relations[n,k]:

    proj[n,k,:] = item_n @ Wh^T + ent_e @ We^T + b          (64-dim)
    e_input[n,k] = proj[n,k,:] . rel_r
    att = softmax_k(leaky_relu(e_input) masked to -9e15 where e == pad)

Because (x @ W^T) . r == x . (r @ W), this factorizes into two scalar tables:

    T[e, r] = ent_e . u_r          u_r = relEmbs[r] @ We_part  (80001 x 40)
    Q[n, r] = item_n . v_r + c_r   v_r = relEmbs[r] @ Wh_part, c_r = b . rel_r

    e_input[n,k] = T[e_nk, r_nk] + Q[n, r_nk]

On device (per core, items sharded 8 ways):
  1. T is computed with one streamed matmul over the transposed entity table
     (uploaded bf16, two 40001-column halves stacked into 128 partitions) and
     written to DRAM as an (80, 40001) f32 table.
  2. Each (item,k) pair gathers its T scalar with indirect DMA: one
     instruction gathers 8192 scalars into one partition's free dim (dest AP
     [1, 8192, 1]; offsets are consumed partition-fastest with wrap), then a
     cheap SBUF->SBUF DMA respreads them onto a [128, 64] tile slice.
  3. Q rows are computed on-chip (PE) from a per-core transposed item table
     and selected per pair with a one-hot compare/mult/reduce on the DVE.
  4. leaky_relu = max(x, 0.2x); mask via min(x, maskv); grouped softmax over
     each item's 32 slots. Output written as a (128, 960) tile per core.
"""

import sys

sys.path.insert(0, "/opt/trn_rl_repo")

import numpy as np
import ml_dtypes

import concourse.bass as bass
import concourse.tile as tile
from concourse import bacc, mybir
from concourse.bass_utils import run_bass_kernel_spmd

# problem constants (hardcoded per harness contract)
N_ITEMS = 30000
K = 32
D = 64
N_ENT = 80000
N_REL = 40
NEG_SLOPE = 0.2
MASK_VAL = -9e15

NCORES = 8
ITEMS_PER_CORE = N_ITEMS // NCORES        # 3750
ITEMS_PAD = 3840                          # 30 chunks of 128
NCHUNKS = ITEMS_PAD // 128                # 30
COLS = NCHUNKS * K                        # 960 free columns in the big tile
HALF = 40001                              # entities per stacked half (A half)
PAIRS = 128 * COLS                        # 122880 gathers per core
GCH = 8192                                # gather descriptors per instruction
GCOLS = GCH // 128                        # 64 big-tile columns per gather
NGCH = PAIRS // GCH                       # 15 gather instructions
TCH = 512                                 # T-pass matmul free-dim chunk
BIGPOS = 3.0e38


def build_program():
    nc = bacc.Bacc("TRN2", debug=False)
    dt = mybir.dt

    entPT2 = nc.dram_tensor("entPT2", [128, HALF], dt.bfloat16, kind="ExternalInput")
    itemT = nc.dram_tensor("itemT", [64, ITEMS_PAD], dt.bfloat16, kind="ExternalInput")
    uT2 = nc.dram_tensor("uT2", [128, 80], dt.bfloat16, kind="ExternalInput")
    vT = nc.dram_tensor("vT", [64, N_REL], dt.bfloat16, kind="ExternalInput")
    crep = nc.dram_tensor("crep", [128, N_REL], dt.float32, kind="ExternalInput")
    idxg = nc.dram_tensor("idxg", [128, COLS], dt.int32, kind="ExternalInput")
    rK = nc.dram_tensor("rK", [128, COLS], dt.float32, kind="ExternalInput")
    maskv = nc.dram_tensor("maskv", [128, COLS], dt.float32, kind="ExternalInput")
    att_out = nc.dram_tensor("att_out", [128, COLS], dt.float32, kind="ExternalOutput")

    ncols_T = [TCH] * (HALF // TCH) + ([HALF % TCH] if HALF % TCH else [])

    with tile.TileContext(nc) as tc:
        import contextlib

        with contextlib.ExitStack() as ctx:
            cpool = ctx.enter_context(tc.tile_pool(name="const", bufs=1))
            tpool = ctx.enter_context(tc.tile_pool(name="tch", bufs=4))
            pp = ctx.enter_context(tc.tile_pool(name="pt", bufs=4, space="PSUM"))
            topool = ctx.enter_context(tc.tile_pool(name="tout", bufs=4))
            qpool = ctx.enter_context(tc.tile_pool(name="q", bufs=2))
            qpp = ctx.enter_context(tc.tile_pool(name="qp", bufs=2, space="PSUM"))
            gpool = ctx.enter_context(tc.tile_pool(name="g", bufs=2))
            dpool = ctx.enter_context(tc.tile_pool(name="dram", bufs=1, space="DRAM"))

            # constant loads
            idx_sb = cpool.tile([128, COLS], dt.int32)
            nc.sync.dma_start(idx_sb[:], idxg[:, :])
            rk_sb = cpool.tile([128, COLS], dt.float32)
            nc.sync.dma_start(rk_sb[:], rK[:, :])
            mask_sb = cpool.tile([128, COLS], dt.float32)
            nc.sync.dma_start(mask_sb[:], maskv[:, :])
            u_sb = cpool.tile([128, 80], dt.bfloat16)
            nc.sync.dma_start(u_sb[:], uT2[:, :])
            v_sb = cpool.tile([64, N_REL], dt.bfloat16)
            nc.sync.dma_start(v_sb[:], vT[:, :])
            c_sb = cpool.tile([128, N_REL], dt.float32)
            nc.sync.dma_start(c_sb[:], crep[:, :])

            iota_sb = cpool.tile([128, K * N_REL], dt.float32)
            nc.gpsimd.iota(
                iota_sb[:], pattern=[[0, K], [1, N_REL]], base=0,
                channel_multiplier=0, allow_small_or_imprecise_dtypes=True,
            )

            big = cpool.tile([128, COLS], dt.float32)
            ex = cpool.tile([128, COLS], dt.float32)
            tmp = cpool.tile([128, COLS], dt.float32)
            qall = cpool.tile([128, N_REL * NCHUNKS], dt.float32)
            oh = cpool.tile([128, K * N_REL], dt.float32)
            qsel = cpool.tile([128, K], dt.float32)
            mx = cpool.tile([128, NCHUNKS], dt.float32)
            sm = cpool.tile([128, NCHUNKS], dt.float32)
            rc = cpool.tile([128, NCHUNKS], dt.float32)

            Td = dpool.tile([80, HALF], dt.float32)

            # ---- T pass: T^T = (uT2)^T @ entPT2, streamed over columns ----
            col = 0
            for w in ncols_T:
                ch = tpool.tile([128, TCH], dt.bfloat16, tag="ch")
                nc.sync.dma_start(ch[:, :w], entPT2[:, col:col + w])
                pt = pp.tile([80, TCH], dt.float32, tag="pt")
                nc.tensor.matmul(
                    out=pt[:, :w], lhsT=u_sb[:], rhs=ch[:, :w],
                    start=True, stop=True,
                )
                to = topool.tile([80, TCH], dt.float32, tag="to")
                nc.scalar.copy(to[:, :w], pt[:, :w])
                nc.sync.dma_start(Td[:, col:col + w], to[:, :w])
                col += w

            # ---- Q pass: per 128-item chunk, Q = items @ V^T + c ----
            for t in range(NCHUNKS):
                lq = qpool.tile([64, 128], dt.bfloat16, tag="lq")
                nc.sync.dma_start(lq[:], itemT[:, t * 128:(t + 1) * 128])
                pq = qpp.tile([128, N_REL], dt.float32, tag="pq")
                nc.tensor.matmul(out=pq[:], lhsT=lq[:], rhs=v_sb[:],
                                 start=True, stop=True)
                nc.vector.tensor_add(
                    qall[:, t * N_REL:(t + 1) * N_REL], pq[:], c_sb[:])

            # ---- gather T scalars ----
            for gi in range(NGCH):
                g = gpool.tile([1, GCH, 1], dt.float32, tag="g")
                nc.gpsimd.indirect_dma_start(
                    out=g[:], out_offset=None,
                    in_=Td[:, :],
                    in_offset=bass.IndirectOffsetOnAxis(
                        ap=idx_sb[:, gi * GCOLS:(gi + 1) * GCOLS], axis=1),
                )
                src = g[:].rearrange("one (p s) unit -> one p (s unit)",
                                     p=128, s=GCOLS)
                nc.sync.dma_start(big[:, gi * GCOLS:(gi + 1) * GCOLS], src)

            # ---- per-chunk Q select and accumulate ----
            for t in range(NCHUNKS):
                rk3 = (rk_sb[:, t * K:(t + 1) * K]
                       .rearrange("p k -> p k ()")
                       .broadcast_to([128, K, N_REL]))
                io3 = iota_sb[:].rearrange("p (k r) -> p k r", k=K)
                nc.vector.tensor_tensor(
                    out=oh[:].rearrange("p (k r) -> p k r", k=K),
                    in0=rk3, in1=io3, op=mybir.AluOpType.is_equal)
                q3 = (qall[:, t * N_REL:(t + 1) * N_REL]
                      .rearrange("p r -> p () r")
                      .broadcast_to([128, K, N_REL]))
                nc.vector.tensor_tensor(
                    out=oh[:].rearrange("p (k r) -> p k r", k=K),
                    in0=oh[:].rearrange("p (k r) -> p k r", k=K),
                    in1=q3, op=mybir.AluOpType.mult)
                nc.vector.tensor_reduce(
                    out=qsel[:], in_=oh[:].rearrange("p (k r) -> p k r", k=K),
                    axis=mybir.AxisListType.X, op=mybir.AluOpType.add)
                nc.vector.tensor_add(
                    big[:, t * K:(t + 1) * K],
                    big[:, t * K:(t + 1) * K], qsel[:])

            # ---- leaky relu, mask, grouped softmax ----
            nc.vector.tensor_scalar_mul(tmp[:], big[:], NEG_SLOPE)
            nc.vector.tensor_tensor(out=big[:], in0=big[:], in1=tmp[:],
                                    op=mybir.AluOpType.max)
            nc.vector.tensor_tensor(out=big[:], in0=big[:], in1=mask_sb[:],
                                    op=mybir.AluOpType.min)

            big3 = big[:].rearrange("p (t k) -> p t k", t=NCHUNKS)
            nc.vector.tensor_reduce(out=mx[:], in_=big3,
                                    axis=mybir.AxisListType.X,
                                    op=mybir.AluOpType.max)
            mx3 = (mx[:].rearrange("p t -> p t ()")
                   .broadcast_to([128, NCHUNKS, K]))
            nc.vector.tensor_tensor(out=big3, in0=big3, in1=mx3,
                                    op=mybir.AluOpType.subtract)
            nc.scalar.activation(out=ex[:], in_=big[:],
                                 func=mybir.ActivationFunctionType.Exp)
            ex3 = ex[:].rearrange("p (t k) -> p t k", t=NCHUNKS)
            nc.vector.tensor_reduce(out=sm[:], in_=ex3,
                                    axis=mybir.AxisListType.X,
                                    op=mybir.AluOpType.add)
            nc.vector.reciprocal(rc[:], sm[:])
            rc3 = (rc[:].rearrange("p t -> p t ()")
                   .broadcast_to([128, NCHUNKS, K]))
            nc.vector.tensor_tensor(out=ex3, in0=ex3, in1=rc3,
                                    op=mybir.AluOpType.mult)
            nc.sync.dma_start(att_out[:, :], ex[:])

    nc.compile()
    return nc


def prep_common(entiEmbs, relEmbs, W_w, W_b):
    d = D
    entP = np.concatenate([np.asarray(entiEmbs, np.float32),
                           np.zeros((1, d), np.float32)], axis=0)  # (80001, 64)
    Wh_part = np.asarray(W_w, np.float32)[:, :d]
    We_part = np.asarray(W_w, np.float32)[:, d:]
    relE = np.asarray(relEmbs, np.float32)
    U = relE @ We_part                      # (40, 64)
    V = relE @ Wh_part                      # (40, 64)
    c = relE @ np.asarray(W_b, np.float32)  # (40,)

    A = entP[:HALF].T                       # (64, 40001)
    Bn = entP[HALF:].T                      # (64, 40000)
    Bp = np.zeros((64, HALF), np.float32)
    Bp[:, :Bn.shape[1]] = Bn
    entPT2 = np.concatenate([A, Bp], axis=0).astype(ml_dtypes.bfloat16)

    uT2 = np.zeros((128, 80), np.float32)
    uT2[0:64, 0:40] = U.T
    uT2[64:128, 40:80] = U.T
    uT2 = uT2.astype(ml_dtypes.bfloat16)
    vT = V.T.astype(ml_dtypes.bfloat16)     # (64, 40)
    crep = np.tile(c[None, :], (128, 1)).astype(np.float32)
    return entP, entPT2, uT2, vT, crep


def canon(arr_core):
    """(3840, 32) -> canonical (128, 960) with column t*32+k = item t*128+p."""
    return (arr_core.reshape(NCHUNKS, 128, K)
            .transpose(1, 0, 2).reshape(128, COLS))


def prep_core(c, entP, item_ids, item_entities, item_relations, hw_order=True):
    lo = c * ITEMS_PER_CORE
    item_ids_shard = np.asarray(item_ids[lo:lo + ITEMS_PER_CORE], np.int64)
    ents = np.zeros((ITEMS_PAD, K), np.int64)
    rels = np.ones((ITEMS_PAD, K), np.int64)
    ents[:ITEMS_PER_CORE] = np.asarray(
        item_entities[lo:lo + ITEMS_PER_CORE], np.int64)
    rels[:ITEMS_PER_CORE] = np.asarray(
        item_relations[lo:lo + ITEMS_PER_CORE], np.int64)
    maskb = np.where(ents != N_ENT, BIGPOS, MASK_VAL).astype(np.float32)
    maskb[ITEMS_PER_CORE:] = MASK_VAL

    r0 = rels - 1
    fidx = np.where(
        ents < HALF,
        r0 * HALF + ents,
        (N_REL + r0) * HALF + (ents - HALF),
    ).astype(np.int32)

    fidx_c = canon(fidx)
    rk_c = canon(r0.astype(np.float32))
    mask_c = canon(maskb)

    if hw_order:
        # descriptor i of gather chunk gi reads offset idx[i % 128, gi*64 + i//128]
        # and its scalar lands at big[i // 64, gi*64 + i % 64]
        idx_up = np.empty((128, COLS), np.int32)
        for gi in range(NGCH):
            F = fidx_c[:, gi * GCOLS:(gi + 1) * GCOLS]      # (128, 64)
            idx_up[:, gi * GCOLS:(gi + 1) * GCOLS] = (
                F.reshape(GCH).reshape(GCOLS, 128).T)
    else:
        idx_up = fidx_c  # CoreSim consumes offsets row-major

    itemsl = np.zeros((ITEMS_PAD, D), np.float32)
    itemsl[:ITEMS_PER_CORE] = entP[item_ids_shard]
    itemT = np.ascontiguousarray(itemsl.T).astype(ml_dtypes.bfloat16)
    return idx_up, rk_c, mask_c, itemT


def make_in_maps(inputs, hw_order=True):
    entP, entPT2, uT2, vT, crep = prep_common(
        inputs["entiEmbs"], inputs["relEmbs"], inputs["W_w"], inputs["W_b"])
    in_maps = []
    for c in range(NCORES):
        idx_up, rk_c, mask_c, itemT = prep_core(
            c, entP, inputs["item_ids"], inputs["item_entities"],
            inputs["item_relations"], hw_order=hw_order)
        in_maps.append({
            "entPT2": entPT2, "itemT": itemT, "uT2": uT2, "vT": vT,
            "crep": crep, "idxg": idx_up, "rK": rk_c, "maskv": mask_c,
        })
    return in_maps


def assemble_output(results):
    out = np.zeros((N_ITEMS, K), np.float32)
    for c in range(NCORES):
        att = results[c]["att_out"]                          # (128, 960)
        arr = (att.reshape(128, NCHUNKS, K).transpose(1, 0, 2)
               .reshape(ITEMS_PAD, K))
        out[c * ITEMS_PER_CORE:(c + 1) * ITEMS_PER_CORE] = arr[:ITEMS_PER_CORE]
    return out


_NC_CACHE = {}


def get_program():
    if "nc" not in _NC_CACHE:
        _NC_CACHE["nc"] = build_program()
    return _NC_CACHE["nc"]


def kernel(entiEmbs, relEmbs, W_w, W_b, item_ids, item_entities,
           item_relations, n_entities):
    inputs = dict(entiEmbs=entiEmbs, relEmbs=relEmbs, W_w=W_w, W_b=W_b,
                  item_ids=item_ids, item_entities=item_entities,
                  item_relations=item_relations, n_entities=n_entities)
    nc = get_program()
    in_maps = make_in_maps(inputs, hw_order=True)
    res = run_bass_kernel_spmd(nc, in_maps, core_ids=list(range(NCORES)))
    return assemble_output(res.results)
